# revision 35
# baseline (speedup 1.0000x reference)
"""Trainium2 Bass kernel for nn_MultiHeadAttention_80418967650946.

Reference computation (per batch b):
  qp/kp/vp = 1x1-conv projections of q/k/v   [64, N]
  funky head view: qh[h,n,d] = qp.reshape(4, 16*N)[d, 16n+h]  (same for kh, vh)
  scores = qh @ kh * 0.25^0.5 + bias ; attn = softmax(scores)
  x[4h+d, n] = (attn @ vh)[h, n, d] ; y = LeakyReLU(BN(Wo @ x + bo), 0.2)

Sharding: 8 cores = 4 batches x 2 query-halves (n in [0,512) or [512,1024)).
Each core computes its query-half for ALL 16 heads fully locally (no
collectives): the output conv is column-wise independent, so y[:, n-half]
only needs x[:, n-half].

Per-core device algorithm (all matmul accum fp32):
  - a ~5us dummy-matmul warm-up chain at t=0 (overlapped with input DMA)
    flips the PE HAM clock gate to 8/8 so real matmuls run at 2.4 GHz;
    a dummy exp at t=0 preloads the ACT exp table set.
  - softmax uses exp(s+b) = exp(s)*exp(b): the host precomputes exp(bias)
    in bf16 (halves HBM traffic vs fp32 bias and removes the f32 psum
    bias-add); the device multiplies exp(scores) by it in the all-bf16
    DVE 2x-rate mode, with alternate tiles offloaded to GpSimd.
  - projections on TensorE produce Kp2 [4, 16384] (d-major, J=16m+h free,
    gathered from psum by f32->bf16 casting GpSimd DMAs, zero engine
    time), Qp2 [4, 8192] (SCALE folded into Wq on the host), and
    Vtm [128, 1024] where per (h, t) the 8 columns are [v0..v3, 1,1,1,1]
    (4 ones columns -> 4 denominator copies in the attn@V psum).
  - scoresT[m-chunk, n] psum tiles come from one K=4 matmul pair packed
    into distinct PE row-groups; exp() runs on ScalarE psum->sbuf bf16.
  - attn@V contracts m on partitions via K=128 matmuls: psum rows 0..3
    are x, rows 4..7 are the softmax denominator; the denominator rows
    are DMA'd to sbuf (partition base 0), reciprocal'd and multiplied on
    DVE -- no partition broadcast needed.
  - heads are software-pipelined and the PE program alternates
    scores(h+1) pairs with attn@V(h) half-chains so the PE FIFO never
    parks ready matmuls behind dependency-stalled ones.
"""
import sys

if "/opt/trn_rl_repo" not in sys.path:
    sys.path.insert(0, "/opt/trn_rl_repo")

import numpy as np
import ml_dtypes

import concourse.bass as bass
import concourse.tile as tile
from concourse import bacc, mybir
from concourse.bass_utils import run_bass_kernel_spmd

F32 = mybir.dt.float32
AF = mybir.ActivationFunctionType
ALU = mybir.AluOpType
PSUM = bass.MemorySpace.PSUM
F32R = mybir.dt.float32r
BF16 = mybir.dt.bfloat16


H = 16
D = 4
HID = 256
B = 4
N = 1024
NH = 512          # per-core query positions
NCORES = 8
SCALE = float(D) ** -0.5
BN_EPS = 1e-5
NEG_SLOPE = 0.2


def _emit(nc, tc, io):
    kb, qb, vb = io["kb"], io["qb"], io["vb"]
    ebT, wkT, wvT, wqT, woT = io["ebT"], io["wkT"], io["wvT"], io["wqT"], io["woT"]
    bnv, y = io["bnv"], io["y"]

    with (
        tc.tile_pool(name="persist", bufs=1) as persist,
        tc.tile_pool(name="bias", bufs=3) as bp,
        tc.tile_pool(name="exp", bufs=4) as ep,
        tc.tile_pool(name="emul", bufs=4) as em,
        tc.tile_pool(name="sml", bufs=4) as sp,
        tc.tile_pool(name="p1", bufs=1) as p1,
        tc.tile_pool(name="ps_s", bufs=3, space=PSUM) as pss,
        tc.tile_pool(name="ps_x", bufs=2, space=PSUM) as psx,
    ):
        Ks = [persist.tile([128, N], BF16, tag=f"Ks{u}", name=f"Ks{u}")
              for u in range(4)]
        Qp2 = persist.tile([100, H * NH], BF16, tag="Qp2")
        Vtm = persist.tile([128, H * 64 + 96], BF16, tag="Vtm")
        x_sb = persist.tile([64, NH], F32R, tag="x_sb")
        woT_sb = persist.tile([64, HID], F32R, tag="woT_sb")
        s_sb = persist.tile([128, 2], F32, tag="s_sb")
        t_sb = persist.tile([128, 2], F32, tag="t_sb")

        # ---- PE warm-up + ACT table preload: no input deps, issue at t=0.
        # The chained matmuls keep the PE busy through a full HAM SHORT
        # window so the clock gate opens to 8/8 before the real matmuls;
        # the dummy exp pulls the ~2.7us exp table load off the critical
        # path.
        wu_w = p1.tile([128, 128], BF16, tag="wu_w")
        wu_r = p1.tile([128, 512], BF16, tag="wu_r")
        nc.vector.memset(wu_w[:], 0.03125)
        nc.vector.memset(wu_r[:], 0.03125)
        scr = p1.tile([128, 8], F32, tag="scr")
        nc.scalar.activation(scr[:], wu_w[:, 0:8], AF.Exp)
        ps_w = pss.tile([128, 512], F32, tag="ps")
        for i in range(12):
            nc.tensor.matmul(ps_w[:], wu_w[:], wu_r[:],
                             start=(i == 0), stop=(i == 11))

        # ---------------- phase 1: projections + BN vectors ----------------
        k_sb = p1.tile([128, 2048], BF16, tag="k_sb")
        q_sb = p1.tile([128, 2048], BF16, tag="q_sb")
        v_sb = p1.tile([128, 2048], BF16, tag="v_sb")
        nc.gpsimd.dma_start(q_sb[:].rearrange("p (c n) -> p c n", c=2),
                            qb.rearrange("(c p) n -> p c n", p=128))
        nc.gpsimd.dma_start(k_sb[:].rearrange("p (c n) -> p c n", c=2),
                            kb.rearrange("(c p) n -> p c n", p=128))
        nc.gpsimd.dma_start(v_sb[:].rearrange("p (c n) -> p c n", c=2),
                            vb.rearrange("(c p) n -> p c n", p=128))
        wk_sb = p1.tile([128, 1024], BF16, tag="wk_sb")
        wv_sb = p1.tile([128, 128], BF16, tag="wv_sb")
        wq_sb = p1.tile([128, 64], BF16, tag="wq_sb")
        nc.gpsimd.dma_start(wq_sb[:].rearrange("p (c o) -> p c o", c=2),
                            wqT.rearrange("(c p) o -> p c o", p=128))
        nc.gpsimd.dma_start(wk_sb[:].rearrange("p (c o) -> p c o", c=2),
                            wkT.rearrange("(c p) o -> p c o", p=128))
        nc.gpsimd.dma_start(wv_sb[:].rearrange("p (c o) -> p c o", c=2),
                            wvT.rearrange("(c p) o -> p c o", p=128))
        nc.gpsimd.dma_start(woT_sb[:], woT)

        # BN affine: s = gamma * rsqrt(var+eps), t = (bo - mean) * s + beta
        bn_sb = p1.tile([128, 10], F32, tag="bn_sb")
        nc.gpsimd.dma_start(bn_sb[:], bnv)

        # ---- exp(bias) prefetch: no dependencies, issue at t=0 ----
        # host layout [H, 128, 8, 512] bf16 = (h, p, t, n): one contiguous
        # 8 KiB read per (partition, head).
        bias_tiles = {}
        for h0 in (0, 2, 4):
            bh2 = bp.tile([128, 8192], BF16, tag="bh2")
            nc.gpsimd.dma_start(
                bh2[:].rearrange("p (h t n) -> p h t n", h=2, t=8),
                ebT[h0:h0 + 2].rearrange("h p t n -> p h t n"))
            bias_tiles[h0] = bh2

        tmp = p1.tile([128, 2], F32, tag="tmp")
        tmp2 = p1.tile([128, 2], F32, tag="tmp2")
        nc.vector.tensor_scalar_add(tmp[:], bn_sb[:, 6:8], BN_EPS)
        nc.scalar.sqrt(tmp[:], tmp[:])
        nc.vector.reciprocal(tmp[:], tmp[:])
        nc.vector.tensor_mul(s_sb[:], bn_sb[:, 0:2], tmp[:])
        nc.vector.tensor_sub(tmp2[:], bn_sb[:, 8:10], bn_sb[:, 4:6])
        nc.vector.tensor_mul(tmp2[:], tmp2[:], s_sb[:])
        nc.vector.tensor_add(t_sb[:], tmp2[:], bn_sb[:, 2:4])

        # Q projection, 4 j-values col-tiled per [128,1024] psum tile
        # (rows 32g+d hold j = 4*b4+g); SCALE is folded into Wq host-side,
        # so the head-major gather into Qp2 is a plain (strided) copy,
        # split across DVE and ScalarE.
        for b4 in range(2):
            psq = pss.tile([128, 1024], F32, tag="ps")
            for g in range(4):
                j = 4 * b4 + g
                for nn2 in range(2):
                    for c in range(2):
                        nc.tensor.matmul(
                            psq[32 * g:32 * g + 4, 512 * nn2:512 * nn2 + 512],
                            wq_sb[:, 32 * c + 4 * j:32 * c + 4 * j + 4],
                            q_sb[:, 1024 * c + 512 * nn2:1024 * c + 512 * nn2 + 512],
                            start=(c == 0), stop=(c == 1), tile_position=(0, 32 * g))
            for g in range(4):
                j = 4 * b4 + g
                srcv = psq[32 * g:32 * g + 4, :].rearrange("d (a b) -> d b a", b=16)
                dstv = Qp2[0:4, :].rearrange("d (b q) -> d b q", b=16)[:, :, 64 * j:64 * j + 64]
                if g % 2 == 0:
                    nc.vector.tensor_copy(dstv, srcv)
                else:
                    nc.scalar.copy(dstv, srcv)
        for rep in range(1, 4):
            nc.sync.dma_start(Qp2[32 * rep:32 * rep + 4, :], Qp2[0:4, :])

        # K projection: M=32 matmuls whose lhsT is host-padded with zero
        # columns, so each psum tile is fully initialized and ONE bulk
        # [100,1024] f32->bf16 copy per tile stages it into Ks[b4]. The
        # four j's of a tile land in the four 32-row groups -- exactly the
        # row-groups the packed scores matmuls read, so no replication is
        # needed.  wk_sb col layout: 512*c2 + 32*j + r  (r<4 real, else 0).
        for b4 in range(4):
            psk = pss.tile([128, 1024], F32, tag="ps")
            for g in range(4):
                j = 4 * b4 + g
                for nn2 in range(2):
                    for c in range(2):
                        nc.tensor.matmul(
                            psk[32 * g:32 * g + 32, 512 * nn2:512 * nn2 + 512],
                            wk_sb[:, 512 * c + 32 * j:512 * c + 32 * j + 32],
                            k_sb[:, 1024 * c + 512 * nn2:1024 * c + 512 * nn2 + 512],
                            start=(c == 0), stop=(c == 1), tile_position=(0, 32 * g))
            if b4 % 2 == 0:
                nc.vector.tensor_copy(Ks[b4][0:100, :], psk[0:100, :])
            else:
                nc.scalar.copy(Ks[b4][0:100, :], psk[0:100, :])

        # V projection into Vtm [128, (h, q64)] bf16:
        #   Vtm[p, 64h + 4t + d]    = vh[m = 128t + p, d]  for head h (q<32)
        #   Vtm[p, 64h + 32 .. 64]  = 1.0
        # The attn@V lhsT for (h, t) is Vtm[:, 64h+4t : 64h+4t+36]: cols
        # 0..3 are v, cols 32..35 are ones -> psum rows 32..35 hold the
        # softmax denominator at a 32-aligned partition base DVE can read.
        nc.vector.memset(Vtm[:], 0.03125)
        for s in range(16):
            psv = psx.tile([64, 64], F32, tag="ps5")
            for c in range(2):
                nc.tensor.matmul(
                    psv[:],
                    v_sb[:, 1024 * c + s:1024 * c + s + 1009:16],
                    wv_sb[:, 64 * c:64 * c + 64],
                    start=(c == 0), stop=(c == 1),
                )
            pv = psv[:].rearrange("r (d c2) -> r d c2", c2=16)
            dst = Vtm[:, 0:H * 64].rearrange("p (h q) -> p h q", q=64)
            dstv = dst[:, :, 0:32].rearrange("p h (t d) -> p h t d", d=4)
            if s % 2 == 0:
                nc.vector.tensor_copy(dstv[0:64, s, :, :],
                                      pv[:, :, 0:16:2].transpose([0, 2, 1]))
                nc.scalar.copy(dstv[64:128, s, :, :],
                               pv[:, :, 1:16:2].transpose([0, 2, 1]))
            else:
                nc.scalar.copy(dstv[0:64, s, :, :],
                               pv[:, :, 0:16:2].transpose([0, 2, 1]))
                nc.vector.tensor_copy(dstv[64:128, s, :, :],
                                      pv[:, :, 1:16:2].transpose([0, 2, 1]))
        ones_f32 = p1.tile([128, 512], F32, tag="ones_f32")
        nc.vector.memset(ones_f32[:], 1.0)
        nc.vector.tensor_copy(
            Vtm[:, 0:H * 64].rearrange("p (h q) -> p h q", q=64)[:, :, 32:64],
            ones_f32[:].rearrange("p (h i) -> p h i", i=32))

        # ---------------- phase 2: attention ----------------
        # Software-pipelined over heads; the PE program per iteration is
        #   scores(h) pair0 | attn@V(h-1) t0..3 | scores(h) pair1 |
        #   attn@V(h-1) t4..7 + normalize(h-1)
        # so the PE FIFO always has ready work and ScalarE stays fed.
        def scores_pair(h, P):
            if P == 0 and h % 2 == 0 and h not in bias_tiles:
                bh2 = bp.tile([128, 8192], BF16, tag="bh2")
                nc.gpsimd.dma_start(
                    bh2[:].rearrange("p (h t n) -> p h t n", h=2, t=8),
                    ebT[h:h + 2].rearrange("h p t n -> p h t n"))
                bias_tiles[h] = bh2
            hb = 4096 * (h % 2)
            bh2 = bias_tiles[h - (h % 2)]
            ex = ep.tile([128, 2048], BF16, tag="ex")
            for uu in range(2):
                u = 2 * P + uu
                # one [128,1024] psum tile = chunks t=2u (cols 0:512) and
                # t=2u+1 (cols 512:); each chunk's two m-halves come from
                # row-groups (2*v2, 2*v2+1) of Ks[u] and go to output
                # partition halves -- all 4 matmuls pack into distinct PE
                # row-groups and run concurrently.
                ps = pss.tile([128, 1024], F32, tag="ps")
                for v2 in range(2):
                    for mh in range(2):
                        rg = 2 * v2 + mh
                        nc.tensor.matmul(
                            ps[64 * mh:64 * mh + 64, 512 * v2:512 * v2 + 512],
                            Ks[u][32 * rg:32 * rg + 4, h:h + 1009:16],
                            Qp2[32 * rg:32 * rg + 4, 512 * h:512 * h + 512],
                            start=True, stop=True,
                            tile_position=(32 * rg, 64 * mh))
                if "dbg_sc" in io and h == 0 and u == 0:
                    dbg_sc_sb = persist.tile([128, 1024], F32, tag="dbg_sc_sb")
                    nc.vector.tensor_copy(dbg_sc_sb[:], ps[:])
                    nc.sync.dma_start(io["dbg_sc"], dbg_sc_sb[:])
                nc.scalar.activation(ex[:, 1024 * uu:1024 * uu + 1024],
                                     ps[:], AF.Exp)
            # exp(s)*exp(b): all-bf16 contiguous -> DVE 2x-rate mode;
            # alternate heads' second tile goes to GpSimd to offload DVE
            exm = em.tile([128, 2048], BF16, tag="exm")
            nc.vector.tensor_mul(exm[:], ex[:],
                                 bh2[:, hb + 2048 * P:hb + 2048 * P + 2048])
            return exm

        def attnv_half(st, half):
            # M=100 (cols 36..99 are junk weights) keeps the PE array's
            # activity monitor fed so the HAM clock gate stays at 8/8;
            # stream time is unchanged (N=512) and rows 36..99 of the same
            # psum bank just accumulate junk.
            h, ems, ps8 = st
            for tt in range(4):
                t = 4 * half + tt
                nc.tensor.matmul(
                    ps8[:],
                    Vtm[:, 64 * h + 4 * t:64 * h + 4 * t + 100],
                    ems[half][:, 512 * tt:512 * tt + 512],
                    start=(t == 0), stop=(t == 7))

        def finish_norm(st):
            h, ems, ps8 = st
            d36 = sp.tile([36, NH], F32, tag="d36")
            nc.vector.tensor_copy(d36[:], ps8[0:36, :])
            d4 = sp.tile([4, NH], F32, tag="d4")
            nc.sync.dma_start(d4[:], d36[32:36, :])
            r4 = sp.tile([4, NH], F32, tag="r4")
            nc.vector.reciprocal_approx_fast(r4[:], d4[:])
            m4 = sp.tile([4, NH], F32R, tag="m4")
            nc.vector.tensor_mul(m4[:], d36[0:4, :], r4[:])
            nc.sync.dma_start(x_sb[4 * h:4 * h + 4, :], m4[:])

        prev = None
        for h in range(H):
            em0 = scores_pair(h, 0)
            if prev is not None:
                attnv_half(prev, 0)
            em1 = scores_pair(h, 1)
            if prev is not None:
                attnv_half(prev, 1)
                finish_norm(prev)
            ps8 = psx.tile([100, NH], F32, tag="ps5")
            prev = (h, [em0, em1], ps8)
        attnv_half(prev, 0)
        attnv_half(prev, 1)
        finish_norm(prev)

        # ---------------- phase 3: output conv + BN + LeakyReLU ----------------
        for u in range(2):
            psy = pss.tile([128, NH], F32, tag="ps")
            nc.tensor.matmul(psy[:], woT_sb[0:64, 128 * u:128 * u + 128], x_sb[:],
                             start=True, stop=True)
            y2 = sp.tile([128, NH], F32, tag="y2")
            nc.vector.tensor_scalar(y2[:], psy[:], s_sb[:, u:u + 1], t_sb[:, u:u + 1],
                                    ALU.mult, ALU.add)
            yt = sp.tile([128, NH], F32, tag="yt")
            nc.vector.scalar_tensor_tensor(yt[:], y2[:], NEG_SLOPE, y2[:],
                                           ALU.mult, ALU.max)
            nc.sync.dma_start(y[128 * u:128 * u + 128, :], yt[:])

        if "dbg_ks" in io:
            nc.sync.dma_start(io["dbg_ks"], Ks[0][:])
            nc.sync.dma_start(io["dbg_vtm"], Vtm[:, 0:H * 64])
            nc.sync.dma_start(io["dbg_x"], x_sb[:])
            nc.sync.dma_start(io["dbg_q"], Qp2[0:100, :])


def build_program(debug_outputs=False):
    nc = bacc.Bacc("TRN2", target_bir_lowering=False, debug=False)
    io = {
        "kb": nc.dram_tensor("kb", [HID, N], BF16, kind="ExternalInput").ap(),
        "qb": nc.dram_tensor("qb", [HID, N], BF16, kind="ExternalInput").ap(),
        "vb": nc.dram_tensor("vb", [HID, N], BF16, kind="ExternalInput").ap(),
        "ebT": nc.dram_tensor("ebT", [H, 128, 8, NH], BF16, kind="ExternalInput").ap(),
        "wkT": nc.dram_tensor("wkT", [HID, 512], BF16, kind="ExternalInput").ap(),
        "wvT": nc.dram_tensor("wvT", [HID, 64], BF16, kind="ExternalInput").ap(),
        "wqT": nc.dram_tensor("wqT", [HID, 32], BF16, kind="ExternalInput").ap(),
        "woT": nc.dram_tensor("woT", [64, HID], F32, kind="ExternalInput").ap(),
        "bnv": nc.dram_tensor("bnv", [128, 10], F32, kind="ExternalInput").ap(),
        "y": nc.dram_tensor("y", [HID, NH], F32, kind="ExternalOutput").ap(),
    }
    if debug_outputs:
        io["dbg_ks"] = nc.dram_tensor("dbg_ks", [128, N], BF16, kind="ExternalOutput").ap()
        io["dbg_vtm"] = nc.dram_tensor("dbg_vtm", [128, H * 64], BF16, kind="ExternalOutput").ap()
        io["dbg_x"] = nc.dram_tensor("dbg_x", [64, NH], F32R, kind="ExternalOutput").ap()
        io["dbg_q"] = nc.dram_tensor("dbg_q", [100, H * NH], BF16, kind="ExternalOutput").ap()
        io["dbg_sc"] = nc.dram_tensor("dbg_sc", [128, 1024], F32, kind="ExternalOutput").ap()
        io["dbg_a36"] = nc.dram_tensor("dbg_a36", [36, NH], F32, kind="ExternalOutput").ap()
    with tile.TileContext(nc) as tc:
        _emit(nc, tc, io)
    nc.compile()
    return nc


def make_in_maps(q, k, v, attn_bias, Wq, Wk, Wv, Wo, bo, gamma, beta, run_mean, run_var):
    def f32(x):
        return np.ascontiguousarray(np.asarray(x, dtype=np.float32))

    def b16(x):
        return np.ascontiguousarray(np.asarray(x, dtype=np.float32).astype(ml_dtypes.bfloat16))

    q, k, v, attn_bias = f32(q), f32(k), f32(v), f32(attn_bias)
    Wq, Wk, Wv, Wo, bo = f32(Wq), f32(Wk), f32(Wv), f32(Wo), f32(bo)
    gamma, beta, run_mean, run_var = f32(gamma), f32(beta), f32(run_mean), f32(run_var)

    # zero-padded K weight layout: col 32*j + r holds Wk row (j + 16*r)
    # for r < 4, zeros elsewhere -> the M=32 projection matmuls fully
    # initialize their psum row-groups.
    wk3 = np.zeros((HID, 512), dtype=np.float32)
    for j in range(16):
        for r in range(4):
            wk3[:, 32 * j + r] = Wk[j + 16 * r, :]
    wkT = b16(wk3)
    wvT = b16(Wv.T)
    woT = f32(Wo.T)
    bnv = np.concatenate(
        [x.reshape(2, 128).T for x in (gamma, beta, run_mean, run_var, bo)], axis=1
    )
    bnv = f32(bnv)

    in_maps = []
    for core in range(NCORES):
        b, half = divmod(core, 2)
        n0 = half * NH
        rows = np.array([16 * d + 8 * half + jl for jl in range(8) for d in range(4)])
        wqT = b16(Wq[rows, :].T * SCALE)                          # [256, 32], col = 4*jl+d
        bt = attn_bias[b, :, n0:n0 + NH, :].transpose(0, 2, 1)          # [16, 1024m, 512n]
        ebT = b16(np.exp(bt.reshape(H, 8, 128, NH).transpose(0, 2, 1, 3)))  # [16, 128p, 8t, 512n]
        in_maps.append({
            "kb": b16(k[b]), "qb": b16(q[b]), "vb": b16(v[b]),
            "ebT": ebT, "wkT": wkT, "wvT": wvT, "wqT": wqT, "woT": woT,
            "bnv": bnv,
        })
    return in_maps


_NC_CACHE = None


def get_nc():
    global _NC_CACHE
    if _NC_CACHE is None:
        _NC_CACHE = build_program()
    return _NC_CACHE


def kernel(**inputs):
    nc = get_nc()
    in_maps = make_in_maps(**inputs)
    res = run_bass_kernel_spmd(nc, in_maps, list(range(NCORES)))
    out = np.empty((B, HID, N), dtype=np.float32)
    for core in range(NCORES):
        b, half = divmod(core, 2)
        out[b, :, half * NH:(half + 1) * NH] = res.results[core]["y"]
    return out


# revision 37
# speedup vs baseline: 1.0248x; 1.0248x over previous
"""Trainium2 Bass kernel for nn_MultiHeadAttention_80418967650946.

Reference computation (per batch b):
  qp/kp/vp = 1x1-conv projections of q/k/v   [64, N]
  funky head view: qh[h,n,d] = qp.reshape(4, 16*N)[d, 16n+h]  (same for kh, vh)
  scores = qh @ kh * 0.25^0.5 + bias ; attn = softmax(scores)
  x[4h+d, n] = (attn @ vh)[h, n, d] ; y = LeakyReLU(BN(Wo @ x + bo), 0.2)

Sharding: 8 cores = 4 batches x 2 query-halves (n in [0,512) or [512,1024)).
Each core computes its query-half for ALL 16 heads fully locally (no
collectives): the output conv is column-wise independent, so y[:, n-half]
only needs x[:, n-half].

Per-core device algorithm (all matmul accum fp32):
  - a ~5us dummy-matmul warm-up chain at t=0 (overlapped with input DMA)
    flips the PE HAM clock gate to 8/8 so real matmuls run at 2.4 GHz;
    a dummy exp at t=0 preloads the ACT exp table set.
  - softmax uses exp(s+b) = exp(s)*exp(b): the host precomputes exp(bias)
    in bf16 (halves HBM traffic vs fp32 bias and removes the f32 psum
    bias-add); the device multiplies exp(scores) by it in the all-bf16
    DVE 2x-rate mode, with alternate tiles offloaded to GpSimd.
  - projections on TensorE produce Kp2 [4, 16384] (d-major, J=16m+h free,
    gathered from psum by f32->bf16 casting GpSimd DMAs, zero engine
    time), Qp2 [4, 8192] (SCALE folded into Wq on the host), and
    Vtm [128, 1024] where per (h, t) the 8 columns are [v0..v3, 1,1,1,1]
    (4 ones columns -> 4 denominator copies in the attn@V psum).
  - scoresT[m-chunk, n] psum tiles come from one K=4 matmul pair packed
    into distinct PE row-groups; exp() runs on ScalarE psum->sbuf bf16.
  - attn@V contracts m on partitions via K=128 matmuls: psum rows 0..3
    are x, rows 4..7 are the softmax denominator; the denominator rows
    are DMA'd to sbuf (partition base 0), reciprocal'd and multiplied on
    DVE -- no partition broadcast needed.
  - heads are software-pipelined and the PE program alternates
    scores(h+1) pairs with attn@V(h) half-chains so the PE FIFO never
    parks ready matmuls behind dependency-stalled ones.
"""
import sys

if "/opt/trn_rl_repo" not in sys.path:
    sys.path.insert(0, "/opt/trn_rl_repo")

import numpy as np
import ml_dtypes

import concourse.bass as bass
import concourse.tile as tile
from concourse import bacc, mybir
from concourse.bass_utils import run_bass_kernel_spmd

F32 = mybir.dt.float32
AF = mybir.ActivationFunctionType
ALU = mybir.AluOpType
PSUM = bass.MemorySpace.PSUM
F32R = mybir.dt.float32r
BF16 = mybir.dt.bfloat16


H = 16
D = 4
HID = 256
B = 4
N = 1024
NH = 512          # per-core query positions
NCORES = 8
SCALE = float(D) ** -0.5
BN_EPS = 1e-5
NEG_SLOPE = 0.2


def _emit(nc, tc, io):
    kb, qb, vb = io["kb"], io["qb"], io["vb"]
    ebT, wkT, wvT, wqT, woT = io["ebT"], io["wkT"], io["wvT"], io["wqT"], io["woT"]
    bnv, y = io["bnv"], io["y"]

    with (
        tc.tile_pool(name="persist", bufs=1) as persist,
        tc.tile_pool(name="bias", bufs=3) as bp,
        tc.tile_pool(name="exp", bufs=4) as ep,
        tc.tile_pool(name="emul", bufs=4) as em,
        tc.tile_pool(name="sml", bufs=4) as sp,
        tc.tile_pool(name="p1", bufs=1) as p1,
        tc.tile_pool(name="ps_s", bufs=3, space=PSUM) as pss,
        tc.tile_pool(name="ps_x", bufs=2, space=PSUM) as psx,
    ):
        Ks = [persist.tile([128, N], BF16, tag=f"Ks{u}", name=f"Ks{u}")
              for u in range(4)]
        Qp2 = persist.tile([100, H * NH], BF16, tag="Qp2")
        Vtm = persist.tile([128, H * 64 + 96], BF16, tag="Vtm")
        x_sb = persist.tile([64, NH], F32R, tag="x_sb")
        woT_sb = persist.tile([64, HID], F32R, tag="woT_sb")
        s_sb = persist.tile([128, 2], F32, tag="s_sb")
        t_sb = persist.tile([128, 2], F32, tag="t_sb")

        # ---- PE warm-up + ACT table preload: no input deps, issue at t=0.
        # The chained matmuls keep the PE busy through a full HAM SHORT
        # window so the clock gate opens to 8/8 before the real matmuls;
        # the dummy exp pulls the ~2.7us exp table load off the critical
        # path.
        wu_w = p1.tile([128, 128], BF16, tag="wu_w")
        wu_r = p1.tile([128, 512], BF16, tag="wu_r")
        nc.vector.memset(wu_w[:], 0.03125)
        nc.vector.memset(wu_r[:], 0.03125)
        scr = p1.tile([128, 8], F32, tag="scr")
        nc.scalar.activation(scr[:], wu_w[:, 0:8], AF.Exp)
        ps_w = pss.tile([128, 512], F32, tag="ps")
        for i in range(8):
            nc.tensor.matmul(ps_w[:], wu_w[:], wu_r[:],
                             start=(i == 0), stop=(i == 7))

        # ---------------- phase 1: projections + BN vectors ----------------
        k_sb = p1.tile([128, 2048], BF16, tag="k_sb")
        q_sb = p1.tile([128, 2048], BF16, tag="q_sb")
        v_sb = p1.tile([128, 2048], BF16, tag="v_sb")
        nc.gpsimd.dma_start(q_sb[:].rearrange("p (c n) -> p c n", c=2),
                            qb.rearrange("(c p) n -> p c n", p=128))
        nc.gpsimd.dma_start(k_sb[:].rearrange("p (c n) -> p c n", c=2),
                            kb.rearrange("(c p) n -> p c n", p=128))
        nc.gpsimd.dma_start(v_sb[:].rearrange("p (c n) -> p c n", c=2),
                            vb.rearrange("(c p) n -> p c n", p=128))
        wk_sb = p1.tile([128, 1024], BF16, tag="wk_sb")
        wv_sb = p1.tile([128, 128], BF16, tag="wv_sb")
        wq_sb = p1.tile([128, 64], BF16, tag="wq_sb")
        nc.gpsimd.dma_start(wq_sb[:].rearrange("p (c o) -> p c o", c=2),
                            wqT.rearrange("(c p) o -> p c o", p=128))
        nc.gpsimd.dma_start(wk_sb[:].rearrange("p (c o) -> p c o", c=2),
                            wkT.rearrange("(c p) o -> p c o", p=128))
        nc.gpsimd.dma_start(wv_sb[:].rearrange("p (c o) -> p c o", c=2),
                            wvT.rearrange("(c p) o -> p c o", p=128))
        nc.gpsimd.dma_start(woT_sb[:], woT)

        # BN affine: s = gamma * rsqrt(var+eps), t = (bo - mean) * s + beta
        bn_sb = p1.tile([128, 10], F32, tag="bn_sb")
        nc.gpsimd.dma_start(bn_sb[:], bnv)

        # ---- exp(bias) prefetch: no dependencies, issue at t=0 ----
        # host layout [H, 128, 8, 512] bf16 = (h, p, t, n): one contiguous
        # 8 KiB read per (partition, head).
        bias_tiles = {}
        for h0 in (0, 2, 4):
            bh2 = bp.tile([128, 8192], BF16, tag="bh2")
            nc.gpsimd.dma_start(
                bh2[:].rearrange("p (h t n) -> p h t n", h=2, t=8),
                ebT[h0:h0 + 2].rearrange("h p t n -> p h t n"))
            bias_tiles[h0] = bh2

        tmp = p1.tile([128, 2], F32, tag="tmp")
        tmp2 = p1.tile([128, 2], F32, tag="tmp2")
        nc.vector.tensor_scalar_add(tmp[:], bn_sb[:, 6:8], BN_EPS)
        nc.scalar.sqrt(tmp[:], tmp[:])
        nc.vector.reciprocal(tmp[:], tmp[:])
        nc.vector.tensor_mul(s_sb[:], bn_sb[:, 0:2], tmp[:])
        nc.vector.tensor_sub(tmp2[:], bn_sb[:, 8:10], bn_sb[:, 4:6])
        nc.vector.tensor_mul(tmp2[:], tmp2[:], s_sb[:])
        nc.vector.tensor_add(t_sb[:], tmp2[:], bn_sb[:, 2:4])

        # Q projection, 4 j-values col-tiled per [128,1024] psum tile
        # (rows 32g+d hold j = 4*b4+g); SCALE is folded into Wq host-side,
        # so the head-major gather into Qp2 is a plain (strided) copy,
        # split across DVE and ScalarE.
        for b4 in range(2):
            psq = pss.tile([128, 1024], F32, tag="ps")
            for g in range(4):
                j = 4 * b4 + g
                for nn2 in range(2):
                    for c in range(2):
                        nc.tensor.matmul(
                            psq[32 * g:32 * g + 4, 512 * nn2:512 * nn2 + 512],
                            wq_sb[:, 32 * c + 4 * j:32 * c + 4 * j + 4],
                            q_sb[:, 1024 * c + 512 * nn2:1024 * c + 512 * nn2 + 512],
                            start=(c == 0), stop=(c == 1), tile_position=(0, 32 * g))
            for g in range(4):
                j = 4 * b4 + g
                srcv = psq[32 * g:32 * g + 4, :].rearrange("d (a b) -> d b a", b=16)
                dstv = Qp2[0:4, :].rearrange("d (b q) -> d b q", b=16)[:, :, 64 * j:64 * j + 64]
                if g % 2 == 0:
                    nc.vector.tensor_copy(dstv, srcv)
                else:
                    nc.scalar.copy(dstv, srcv)
        for rep in range(1, 4):
            nc.sync.dma_start(Qp2[32 * rep:32 * rep + 4, :], Qp2[0:4, :])

        # K projection: M=32 matmuls whose lhsT is host-padded with zero
        # columns, so each psum tile is fully initialized and ONE bulk
        # [100,1024] f32->bf16 copy per tile stages it into Ks[b4]. The
        # four j's of a tile land in the four 32-row groups -- exactly the
        # row-groups the packed scores matmuls read, so no replication is
        # needed.  wk_sb col layout: 512*c2 + 32*j + r  (r<4 real, else 0).
        for b4 in (0, 1):
            psk = pss.tile([128, 1024], F32, tag="ps")
            for g in range(4):
                j = 4 * b4 + g
                for nn2 in range(2):
                    for c in range(2):
                        nc.tensor.matmul(
                            psk[32 * g:32 * g + 32, 512 * nn2:512 * nn2 + 512],
                            wk_sb[:, 512 * c + 32 * j:512 * c + 32 * j + 32],
                            k_sb[:, 1024 * c + 512 * nn2:1024 * c + 512 * nn2 + 512],
                            start=(c == 0), stop=(c == 1), tile_position=(0, 32 * g))
            if b4 % 2 == 0:
                nc.vector.tensor_copy(Ks[b4][0:100, :], psk[0:100, :])
            else:
                nc.scalar.copy(Ks[b4][0:100, :], psk[0:100, :])

        def scores_pair(h, P):
            if P == 0 and h % 2 == 0 and h not in bias_tiles:
                bh2 = bp.tile([128, 8192], BF16, tag="bh2")
                nc.gpsimd.dma_start(
                    bh2[:].rearrange("p (h t n) -> p h t n", h=2, t=8),
                    ebT[h:h + 2].rearrange("h p t n -> p h t n"))
                bias_tiles[h] = bh2
            hb = 4096 * (h % 2)
            bh2 = bias_tiles[h - (h % 2)]
            ex = ep.tile([128, 2048], BF16, tag="ex")
            for uu in range(2):
                u = 2 * P + uu
                # one [128,1024] psum tile = chunks t=2u (cols 0:512) and
                # t=2u+1 (cols 512:); each chunk's two m-halves come from
                # row-groups (2*v2, 2*v2+1) of Ks[u] and go to output
                # partition halves -- all 4 matmuls pack into distinct PE
                # row-groups and run concurrently.
                ps = pss.tile([128, 1024], F32, tag="ps")
                for v2 in range(2):
                    for mh in range(2):
                        rg = 2 * v2 + mh
                        nc.tensor.matmul(
                            ps[64 * mh:64 * mh + 64, 512 * v2:512 * v2 + 512],
                            Ks[u][32 * rg:32 * rg + 4, h:h + 1009:16],
                            Qp2[32 * rg:32 * rg + 4, 512 * h:512 * h + 512],
                            start=True, stop=True,
                            tile_position=(32 * rg, 64 * mh))
                if "dbg_sc" in io and h == 0 and u == 0:
                    dbg_sc_sb = persist.tile([128, 1024], F32, tag="dbg_sc_sb")
                    nc.vector.tensor_copy(dbg_sc_sb[:], ps[:])
                    nc.sync.dma_start(io["dbg_sc"], dbg_sc_sb[:])
                nc.scalar.activation(ex[:, 1024 * uu:1024 * uu + 1024],
                                     ps[:], AF.Exp)
            # exp(s)*exp(b): all-bf16 contiguous -> DVE 2x-rate mode;
            # alternate heads' second tile goes to GpSimd to offload DVE
            exm = em.tile([128, 2048], BF16, tag="exm")
            nc.vector.tensor_mul(exm[:], ex[:],
                                 bh2[:, hb + 2048 * P:hb + 2048 * P + 2048])
            return exm

        def attnv_half(st, half):
            # M=100 (cols 36..99 are junk weights) keeps the PE array's
            # activity monitor fed so the HAM clock gate stays at 8/8;
            # stream time is unchanged (N=512) and rows 36..99 of the same
            # psum bank just accumulate junk.
            h, ems, ps8 = st
            for tt in range(4):
                t = 4 * half + tt
                nc.tensor.matmul(
                    ps8[:],
                    Vtm[:, 64 * h + 4 * t:64 * h + 4 * t + 36],
                    ems[half][:, 512 * tt:512 * tt + 512],
                    start=(t == 0), stop=(t == 7))

        def finish_norm(st):
            h, ems, ps8 = st
            d36 = sp.tile([36, NH], F32, tag="d36")
            nc.vector.tensor_copy(d36[:], ps8[:])
            d4 = sp.tile([4, NH], F32, tag="d4")
            nc.sync.dma_start(d4[:], d36[32:36, :])
            r4 = sp.tile([4, NH], F32, tag="r4")
            nc.vector.reciprocal_approx_fast(r4[:], d4[:])
            m4 = sp.tile([4, NH], F32R, tag="m4")
            nc.vector.tensor_mul(m4[:], d36[0:4, :], r4[:])
            nc.sync.dma_start(x_sb[4 * h:4 * h + 4, :], m4[:])

        em0_first = scores_pair(0, 0)

        # V projection into Vtm [128, (h, q64)] bf16:
        #   Vtm[p, 64h + 4t + d]    = vh[m = 128t + p, d]  for head h (q<32)
        #   Vtm[p, 64h + 32 .. 64]  = 1.0
        # The attn@V lhsT for (h, t) is Vtm[:, 64h+4t : 64h+4t+36]: cols
        # 0..3 are v, cols 32..35 are ones -> psum rows 32..35 hold the
        # softmax denominator at a 32-aligned partition base DVE can read.
        nc.vector.memset(Vtm[:], 0.03125)
        for s in range(16):
            psv = psx.tile([64, 64], F32, tag="ps5")
            for c in range(2):
                nc.tensor.matmul(
                    psv[:],
                    v_sb[:, 1024 * c + s:1024 * c + s + 1009:16],
                    wv_sb[:, 64 * c:64 * c + 64],
                    start=(c == 0), stop=(c == 1),
                )
            pv = psv[:].rearrange("r (d c2) -> r d c2", c2=16)
            dst = Vtm[:, 0:H * 64].rearrange("p (h q) -> p h q", q=64)
            dstv = dst[:, :, 0:32].rearrange("p h (t d) -> p h t d", d=4)
            if s % 2 == 0:
                nc.vector.tensor_copy(dstv[0:64, s, :, :],
                                      pv[:, :, 0:16:2].transpose([0, 2, 1]))
                nc.scalar.copy(dstv[64:128, s, :, :],
                               pv[:, :, 1:16:2].transpose([0, 2, 1]))
            else:
                nc.scalar.copy(dstv[0:64, s, :, :],
                               pv[:, :, 0:16:2].transpose([0, 2, 1]))
                nc.vector.tensor_copy(dstv[64:128, s, :, :],
                                      pv[:, :, 1:16:2].transpose([0, 2, 1]))
        ones_f32 = p1.tile([128, 512], F32, tag="ones_f32")
        nc.vector.memset(ones_f32[:], 1.0)
        nc.vector.tensor_copy(
            Vtm[:, 0:H * 64].rearrange("p (h q) -> p h q", q=64)[:, :, 32:64],
            ones_f32[:].rearrange("p (h i) -> p h i", i=32))

        for b4 in (2, 3):
            psk = pss.tile([128, 1024], F32, tag="ps")
            for g in range(4):
                j = 4 * b4 + g
                for nn2 in range(2):
                    for c in range(2):
                        nc.tensor.matmul(
                            psk[32 * g:32 * g + 32, 512 * nn2:512 * nn2 + 512],
                            wk_sb[:, 512 * c + 32 * j:512 * c + 32 * j + 32],
                            k_sb[:, 1024 * c + 512 * nn2:1024 * c + 512 * nn2 + 512],
                            start=(c == 0), stop=(c == 1), tile_position=(0, 32 * g))
            if b4 % 2 == 0:
                nc.vector.tensor_copy(Ks[b4][0:100, :], psk[0:100, :])
            else:
                nc.scalar.copy(Ks[b4][0:100, :], psk[0:100, :])

        # ---------------- phase 2: attention ----------------
        # Software-pipelined over heads; the PE program per iteration is
        #   scores(h) pair0 | attn@V(h-1) t0..3 | scores(h) pair1 |
        #   attn@V(h-1) t4..7 + normalize(h-1)
        # so the PE FIFO always has ready work and ScalarE stays fed.
        prev = None
        for h in range(H):
            em0 = em0_first if h == 0 else scores_pair(h, 0)
            if prev is not None:
                attnv_half(prev, 0)
            em1 = scores_pair(h, 1)
            if prev is not None:
                attnv_half(prev, 1)
                finish_norm(prev)
            ps8 = psx.tile([36, NH], F32, tag="ps5")
            prev = (h, [em0, em1], ps8)
        attnv_half(prev, 0)
        attnv_half(prev, 1)
        finish_norm(prev)

        # ---------------- phase 3: output conv + BN + LeakyReLU ----------------
        for u in range(2):
            psy = pss.tile([128, NH], F32, tag="ps")
            nc.tensor.matmul(psy[:], woT_sb[0:64, 128 * u:128 * u + 128], x_sb[:],
                             start=True, stop=True)
            y2 = sp.tile([128, NH], F32, tag="y2")
            nc.vector.tensor_scalar(y2[:], psy[:], s_sb[:, u:u + 1], t_sb[:, u:u + 1],
                                    ALU.mult, ALU.add)
            yt = sp.tile([128, NH], F32, tag="yt")
            nc.vector.scalar_tensor_tensor(yt[:], y2[:], NEG_SLOPE, y2[:],
                                           ALU.mult, ALU.max)
            nc.sync.dma_start(y[128 * u:128 * u + 128, :], yt[:])

        if "dbg_ks" in io:
            nc.sync.dma_start(io["dbg_ks"], Ks[0][:])
            nc.sync.dma_start(io["dbg_vtm"], Vtm[:, 0:H * 64])
            nc.sync.dma_start(io["dbg_x"], x_sb[:])
            nc.sync.dma_start(io["dbg_q"], Qp2[0:100, :])


def build_program(debug_outputs=False):
    nc = bacc.Bacc("TRN2", target_bir_lowering=False, debug=False)
    io = {
        "kb": nc.dram_tensor("kb", [HID, N], BF16, kind="ExternalInput").ap(),
        "qb": nc.dram_tensor("qb", [HID, N], BF16, kind="ExternalInput").ap(),
        "vb": nc.dram_tensor("vb", [HID, N], BF16, kind="ExternalInput").ap(),
        "ebT": nc.dram_tensor("ebT", [H, 128, 8, NH], BF16, kind="ExternalInput").ap(),
        "wkT": nc.dram_tensor("wkT", [HID, 512], BF16, kind="ExternalInput").ap(),
        "wvT": nc.dram_tensor("wvT", [HID, 64], BF16, kind="ExternalInput").ap(),
        "wqT": nc.dram_tensor("wqT", [HID, 32], BF16, kind="ExternalInput").ap(),
        "woT": nc.dram_tensor("woT", [64, HID], F32, kind="ExternalInput").ap(),
        "bnv": nc.dram_tensor("bnv", [128, 10], F32, kind="ExternalInput").ap(),
        "y": nc.dram_tensor("y", [HID, NH], F32, kind="ExternalOutput").ap(),
    }
    if debug_outputs:
        io["dbg_ks"] = nc.dram_tensor("dbg_ks", [128, N], BF16, kind="ExternalOutput").ap()
        io["dbg_vtm"] = nc.dram_tensor("dbg_vtm", [128, H * 64], BF16, kind="ExternalOutput").ap()
        io["dbg_x"] = nc.dram_tensor("dbg_x", [64, NH], F32R, kind="ExternalOutput").ap()
        io["dbg_q"] = nc.dram_tensor("dbg_q", [100, H * NH], BF16, kind="ExternalOutput").ap()
        io["dbg_sc"] = nc.dram_tensor("dbg_sc", [128, 1024], F32, kind="ExternalOutput").ap()
        io["dbg_a36"] = nc.dram_tensor("dbg_a36", [36, NH], F32, kind="ExternalOutput").ap()
    with tile.TileContext(nc) as tc:
        _emit(nc, tc, io)
    nc.compile()
    return nc


def make_in_maps(q, k, v, attn_bias, Wq, Wk, Wv, Wo, bo, gamma, beta, run_mean, run_var):
    def f32(x):
        return np.ascontiguousarray(np.asarray(x, dtype=np.float32))

    def b16(x):
        return np.ascontiguousarray(np.asarray(x, dtype=np.float32).astype(ml_dtypes.bfloat16))

    q, k, v, attn_bias = f32(q), f32(k), f32(v), f32(attn_bias)
    Wq, Wk, Wv, Wo, bo = f32(Wq), f32(Wk), f32(Wv), f32(Wo), f32(bo)
    gamma, beta, run_mean, run_var = f32(gamma), f32(beta), f32(run_mean), f32(run_var)

    # zero-padded K weight layout: col 32*j + r holds Wk row (j + 16*r)
    # for r < 4, zeros elsewhere -> the M=32 projection matmuls fully
    # initialize their psum row-groups.
    wk3 = np.zeros((HID, 512), dtype=np.float32)
    for j in range(16):
        for r in range(4):
            wk3[:, 32 * j + r] = Wk[j + 16 * r, :]
    wkT = b16(wk3)
    wvT = b16(Wv.T)
    woT = f32(Wo.T)
    bnv = np.concatenate(
        [x.reshape(2, 128).T for x in (gamma, beta, run_mean, run_var, bo)], axis=1
    )
    bnv = f32(bnv)

    in_maps = []
    for core in range(NCORES):
        b, half = divmod(core, 2)
        n0 = half * NH
        rows = np.array([16 * d + 8 * half + jl for jl in range(8) for d in range(4)])
        wqT = b16(Wq[rows, :].T * SCALE)                          # [256, 32], col = 4*jl+d
        bt = attn_bias[b, :, n0:n0 + NH, :].transpose(0, 2, 1)          # [16, 1024m, 512n]
        ebT = b16(np.exp(bt.reshape(H, 8, 128, NH).transpose(0, 2, 1, 3)))  # [16, 128p, 8t, 512n]
        in_maps.append({
            "kb": b16(k[b]), "qb": b16(q[b]), "vb": b16(v[b]),
            "ebT": ebT, "wkT": wkT, "wvT": wvT, "wqT": wqT, "woT": woT,
            "bnv": bnv,
        })
    return in_maps


_NC_CACHE = None


def get_nc():
    global _NC_CACHE
    if _NC_CACHE is None:
        _NC_CACHE = build_program()
    return _NC_CACHE


def kernel(**inputs):
    nc = get_nc()
    in_maps = make_in_maps(**inputs)
    res = run_bass_kernel_spmd(nc, in_maps, list(range(NCORES)))
    out = np.empty((B, HID, N), dtype=np.float32)
    for core in range(NCORES):
        b, half = divmod(core, 2)
        out[b, :, half * NH:(half + 1) * NH] = res.results[core]["y"]
    return out


# revision 39
# speedup vs baseline: 1.0429x; 1.0177x over previous
"""Trainium2 Bass kernel for nn_MultiHeadAttention_80418967650946.

Reference computation (per batch b):
  qp/kp/vp = 1x1-conv projections of q/k/v   [64, N]
  funky head view: qh[h,n,d] = qp.reshape(4, 16*N)[d, 16n+h]  (same for kh, vh)
  scores = qh @ kh * 0.25^0.5 + bias ; attn = softmax(scores)
  x[4h+d, n] = (attn @ vh)[h, n, d] ; y = LeakyReLU(BN(Wo @ x + bo), 0.2)

Sharding: 8 cores = 4 batches x 2 query-halves (n in [0,512) or [512,1024)).
Each core computes its query-half for ALL 16 heads fully locally (no
collectives): the output conv is column-wise independent, so y[:, n-half]
only needs x[:, n-half].

Per-core device algorithm (all matmul accum fp32):
  - a dummy-matmul warm-up chain at t=0 (overlapped with input DMA) nudges
    the PE HAM clock gate toward 8/8; a dummy exp at t=0 preloads the ACT
    exp table set off the critical path.
  - softmax uses exp(s+b) = exp(s)*exp(b): the host precomputes exp(bias)
    in bf16 (halves HBM traffic vs fp32 bias and removes the f32 psum
    bias-add); the device multiplies exp(scores) by it in the all-bf16
    DVE 2x-rate mode.
  - K projection runs as M=32 matmuls with host-zero-padded weights so
    each psum tile is fully initialized; ONE bulk [100,1024] f32->bf16
    copy per tile stages it into Ks[b4], whose four 32-row groups are
    exactly what the 4-way-packed scores matmuls read (no replication).
  - scores: per [128,1024] psum tile, 4 matmuls (2 key-chunks x 2
    m-halves) pack into distinct PE row/col groups and run concurrently;
    exp() runs on ScalarE psum->sbuf bf16, N=1024 per instruction.
  - attn@V contracts m on partitions via K=128 matmuls whose lhsT window
    over Vtm carries v (cols 0..3) and ones (cols 32..35): psum rows 0..3
    are x, rows 32..35 the softmax denominator.  The denominator must
    reach partition base 0 for DVE (reciprocal_approx_fast mis-executes
    at partition base 32), so a DVE copy + tiny sbuf->sbuf DMA move it.
  - heads are software-pipelined: the PE program alternates scores(h)
    pairs with attn@V(h-1) half-chains so the PE FIFO always has ready
    work and ScalarE stays fed.
"""
import sys

if "/opt/trn_rl_repo" not in sys.path:
    sys.path.insert(0, "/opt/trn_rl_repo")

import numpy as np
import ml_dtypes

import concourse.bass as bass
import concourse.tile as tile
from concourse import bacc, mybir
from concourse.bass_utils import run_bass_kernel_spmd

F32 = mybir.dt.float32
AF = mybir.ActivationFunctionType
ALU = mybir.AluOpType
PSUM = bass.MemorySpace.PSUM
F32R = mybir.dt.float32r
BF16 = mybir.dt.bfloat16


H = 16
D = 4
HID = 256
B = 4
N = 1024
NH = 512          # per-core query positions
NCORES = 8
SCALE = float(D) ** -0.5
BN_EPS = 1e-5
NEG_SLOPE = 0.2


def _emit(nc, tc, io):
    kb, qb, vb = io["kb"], io["qb"], io["vb"]
    ebT, wkT, wvT, wqT, woT = io["ebT"], io["wkT"], io["wvT"], io["wqT"], io["woT"]
    bnv, y = io["bnv"], io["y"]

    with (
        tc.tile_pool(name="persist", bufs=1) as persist,
        tc.tile_pool(name="bias", bufs=3) as bp,
        tc.tile_pool(name="exp", bufs=4) as ep,
        tc.tile_pool(name="emul", bufs=4) as em,
        tc.tile_pool(name="sml", bufs=4) as sp,
        tc.tile_pool(name="p1", bufs=1) as p1,
        tc.tile_pool(name="ps_s", bufs=3, space=PSUM) as pss,
        tc.tile_pool(name="ps_x", bufs=2, space=PSUM) as psx,
    ):
        Ks = [persist.tile([128, N], BF16, tag=f"Ks{u}", name=f"Ks{u}")
              for u in range(4)]
        Qp2 = persist.tile([100, H * NH], BF16, tag="Qp2")
        Vtm = persist.tile([128, H * 64 + 96], BF16, tag="Vtm")
        x_sb = persist.tile([64, NH], F32R, tag="x_sb")
        woT_sb = persist.tile([64, HID], F32R, tag="woT_sb")
        s_sb = persist.tile([128, 2], F32, tag="s_sb")
        t_sb = persist.tile([128, 2], F32, tag="t_sb")

        # ---- PE warm-up + ACT table preload: no input deps, issue at t=0.
        wu_w = p1.tile([128, 128], BF16, tag="wu_w")
        wu_r = p1.tile([128, 512], BF16, tag="wu_r")
        nc.vector.memset(wu_w[:], 0.03125)
        nc.vector.memset(wu_r[:], 0.03125)
        scr = p1.tile([128, 8], F32, tag="scr")
        nc.scalar.activation(scr[:], wu_w[:, 0:8], AF.Exp)
        ps_w = pss.tile([128, 512], F32, tag="ps")
        for i in range(8):
            nc.tensor.matmul(ps_w[:], wu_w[:], wu_r[:],
                             start=(i == 0), stop=(i == 7))

        # ---------------- phase 1: input DMAs ----------------
        k_sb = p1.tile([128, 2048], BF16, tag="k_sb")
        q_sb = p1.tile([128, 2048], BF16, tag="q_sb")
        v_sb = p1.tile([128, 2048], BF16, tag="v_sb")
        nc.gpsimd.dma_start(q_sb[:].rearrange("p (c n) -> p c n", c=2),
                            qb.rearrange("(c p) n -> p c n", p=128))
        nc.gpsimd.dma_start(k_sb[:].rearrange("p (c n) -> p c n", c=2),
                            kb.rearrange("(c p) n -> p c n", p=128))
        nc.gpsimd.dma_start(v_sb[:].rearrange("p (c n) -> p c n", c=2),
                            vb.rearrange("(c p) n -> p c n", p=128))
        wk_sb = p1.tile([128, 1024], BF16, tag="wk_sb")
        wv_sb = p1.tile([128, 128], BF16, tag="wv_sb")
        wq_sb = p1.tile([128, 64], BF16, tag="wq_sb")
        nc.gpsimd.dma_start(wq_sb[:].rearrange("p (c o) -> p c o", c=2),
                            wqT.rearrange("(c p) o -> p c o", p=128))
        nc.gpsimd.dma_start(wk_sb[:].rearrange("p (c o) -> p c o", c=2),
                            wkT.rearrange("(c p) o -> p c o", p=128))
        nc.gpsimd.dma_start(wv_sb[:].rearrange("p (c o) -> p c o", c=2),
                            wvT.rearrange("(c p) o -> p c o", p=128))
        nc.gpsimd.dma_start(woT_sb[:], woT)

        bn_sb = p1.tile([128, 10], F32, tag="bn_sb")
        nc.gpsimd.dma_start(bn_sb[:], bnv)

        # exp(bias) prefetch: [H, 128, 8, 512] bf16 -> one contiguous 8 KiB
        # read per (partition, head)
        bias_tiles = {}
        for h0 in (0, 2, 4):
            bh2 = bp.tile([128, 8192], BF16, tag="bh2")
            nc.gpsimd.dma_start(
                bh2[:].rearrange("p (h t n) -> p h t n", h=2, t=8),
                ebT[h0:h0 + 2].rearrange("h p t n -> p h t n"))
            bias_tiles[h0] = bh2

        # BN affine: s = gamma * rsqrt(var+eps), t = (bo - mean) * s + beta
        tmp = p1.tile([128, 2], F32, tag="tmp")
        tmp2 = p1.tile([128, 2], F32, tag="tmp2")
        nc.vector.tensor_scalar_add(tmp[:], bn_sb[:, 6:8], BN_EPS)
        nc.scalar.sqrt(tmp[:], tmp[:])
        nc.vector.reciprocal(tmp[:], tmp[:])
        nc.vector.tensor_mul(s_sb[:], bn_sb[:, 0:2], tmp[:])
        nc.vector.tensor_sub(tmp2[:], bn_sb[:, 8:10], bn_sb[:, 4:6])
        nc.vector.tensor_mul(tmp2[:], tmp2[:], s_sb[:])
        nc.vector.tensor_add(t_sb[:], tmp2[:], bn_sb[:, 2:4])

        # ---------------- Q projection ----------------
        # 4 j-values col-tiled per [128,1024] psum tile (rows 32g+d hold
        # j = 4*b4+g); SCALE is folded into Wq host-side so the head-major
        # gather into Qp2 is a plain strided copy, split DVE/ScalarE.
        for b4 in range(2):
            psq = pss.tile([128, 1024], F32, tag="ps")
            for g in range(4):
                j = 4 * b4 + g
                for nn2 in range(2):
                    for c in range(2):
                        nc.tensor.matmul(
                            psq[32 * g:32 * g + 4, 512 * nn2:512 * nn2 + 512],
                            wq_sb[:, 32 * c + 4 * j:32 * c + 4 * j + 4],
                            q_sb[:, 1024 * c + 512 * nn2:1024 * c + 512 * nn2 + 512],
                            start=(c == 0), stop=(c == 1), tile_position=(0, 32 * g))
            for g in range(4):
                j = 4 * b4 + g
                srcv = psq[32 * g:32 * g + 4, :].rearrange("d (a b) -> d b a", b=16)
                dstv = Qp2[0:4, :].rearrange("d (b q) -> d b q", b=16)[:, :, 64 * j:64 * j + 64]
                if g % 2 == 0:
                    nc.vector.tensor_copy(dstv, srcv)
                else:
                    nc.scalar.copy(dstv, srcv)
        for rep in range(1, 4):
            nc.sync.dma_start(Qp2[32 * rep:32 * rep + 4, :], Qp2[0:4, :])

        # ---------------- K projection ----------------
        # M=32 matmuls with host-zero-padded lhsT -> fully-initialized psum,
        # one bulk [100,1024] f32->bf16 copy stages tile b4 into Ks[b4].
        # wk_sb col layout: 512*c2 + 32*j + r  (r<4 real, else 0).
        for b4 in range(4):
            psk = pss.tile([128, 1024], F32, tag="ps")
            for g in range(4):
                j = 4 * b4 + g
                for nn2 in range(2):
                    for c in range(2):
                        nc.tensor.matmul(
                            psk[32 * g:32 * g + 32, 512 * nn2:512 * nn2 + 512],
                            wk_sb[:, 512 * c + 32 * j:512 * c + 32 * j + 32],
                            k_sb[:, 1024 * c + 512 * nn2:1024 * c + 512 * nn2 + 512],
                            start=(c == 0), stop=(c == 1), tile_position=(0, 32 * g))
            if b4 % 2 == 0:
                nc.vector.tensor_copy(Ks[b4][0:100, :], psk[0:100, :])
            else:
                nc.scalar.copy(Ks[b4][0:100, :], psk[0:100, :])

        # ---------------- phase 2 stage functions ----------------
        def scores_pair(h, P):
            if P == 0 and h % 2 == 0 and h not in bias_tiles:
                bh2 = bp.tile([128, 8192], BF16, tag="bh2")
                nc.gpsimd.dma_start(
                    bh2[:].rearrange("p (h t n) -> p h t n", h=2, t=8),
                    ebT[h:h + 2].rearrange("h p t n -> p h t n"))
                bias_tiles[h] = bh2
            hb = 4096 * (h % 2)
            bh2 = bias_tiles[h - (h % 2)]
            ex = ep.tile([128, 2048], BF16, tag="ex")
            for uu in range(2):
                u = 2 * P + uu
                # one [128,1024] psum tile = chunks t=2u (cols 0:512) and
                # t=2u+1 (cols 512:); each chunk's two m-halves come from
                # row-groups (2*v2, 2*v2+1) of Ks[u] and land in output
                # partition halves -- all 4 matmuls pack into distinct PE
                # row/col groups and run concurrently.
                ps = pss.tile([128, 1024], F32, tag="ps")
                for v2 in range(2):
                    for mh in range(2):
                        rg = 2 * v2 + mh
                        nc.tensor.matmul(
                            ps[64 * mh:64 * mh + 64, 512 * v2:512 * v2 + 512],
                            Ks[u][32 * rg:32 * rg + 4, h:h + 1009:16],
                            Qp2[32 * rg:32 * rg + 4, 512 * h:512 * h + 512],
                            start=True, stop=True,
                            tile_position=(32 * rg, 64 * mh))
                nc.scalar.activation(ex[:, 1024 * uu:1024 * uu + 1024],
                                     ps[:], AF.Exp)
            # exp(s)*exp(b): all-bf16 contiguous -> DVE 2x-rate mode
            exm = em.tile([128, 2048], BF16, tag="exm")
            nc.vector.tensor_mul(exm[:], ex[:],
                                 bh2[:, hb + 2048 * P:hb + 2048 * P + 2048])
            return exm

        def attnv_half(st, half):
            h, ems, ps8 = st
            for tt in range(4):
                t = 4 * half + tt
                nc.tensor.matmul(
                    ps8[:],
                    Vtm[:, 64 * h + 4 * t:64 * h + 4 * t + 36],
                    ems[half][:, 512 * tt:512 * tt + 512],
                    start=(t == 0), stop=(t == 7))

        def finish_norm(st):
            h, ems, ps8 = st
            d36 = sp.tile([36, NH], F32, tag="d36")
            nc.vector.tensor_copy(d36[:], ps8[:])
            d4 = sp.tile([4, NH], F32, tag="d4")
            nc.sync.dma_start(d4[:], d36[32:36, :])
            r4 = sp.tile([4, NH], F32, tag="r4")
            nc.vector.reciprocal_approx_fast(r4[:], d4[:])
            m4 = sp.tile([4, NH], F32R, tag="m4")
            nc.vector.tensor_mul(m4[:], d36[0:4, :], r4[:])
            nc.sync.dma_start(x_sb[4 * h:4 * h + 4, :], m4[:])

        # first scores pair before the V projection: ScalarE's EXP stream
        # starts as soon as Ks/Qp2 are staged instead of after V.
        em0_first = scores_pair(0, 0)

        # ---------------- V projection ----------------
        # Vtm [128, (h, q64)] bf16:
        #   Vtm[p, 64h + 4t + d]    = vh[m = 128t + p, d]  for head h
        #   Vtm[p, 64h + 32 .. 64]  = 1.0
        # Host supplies wv2 with cols (c2, d) so the per-head gather below
        # reads 4-element contiguous runs; two heads share one [128,64]
        # psum tile (output col-strips 0/64) to double the ring depth.
        nc.vector.memset(Vtm[:], 0.03125)
        for s2 in range(8):
            psv = psx.tile([128, 64], F32, tag="ps5")
            for half in range(2):
                for c in range(2):
                    nc.tensor.matmul(
                        psv[64 * half:64 * half + 64, :],
                        v_sb[:, 1024 * c + 2 * s2 + half:1024 * c + 2 * s2 + half + 1009:16],
                        wv_sb[:, 64 * c:64 * c + 64],
                        start=(c == 0), stop=(c == 1),
                        tile_position=(0, 64 * half))
            dst = Vtm[:, 0:H * 64].rearrange("p (h q) -> p h q", q=64)
            for half in range(2):
                s = 2 * s2 + half
                dstv = dst[:, s, 0:32].rearrange("p (t d) -> p t d", d=4)
                pv = psv[64 * half:64 * half + 64, :].rearrange(
                    "r (c2 d) -> r c2 d", d=4)
                if half == 0:
                    nc.vector.tensor_copy(dstv[0:64, :, :], pv[:, 0:16:2, :])
                    nc.scalar.copy(dstv[64:128, :, :], pv[:, 1:16:2, :])
                else:
                    nc.scalar.copy(dstv[0:64, :, :], pv[:, 0:16:2, :])
                    nc.vector.tensor_copy(dstv[64:128, :, :], pv[:, 1:16:2, :])
        ones_f32 = p1.tile([128, 512], F32, tag="ones_f32")
        nc.vector.memset(ones_f32[:], 1.0)
        nc.vector.tensor_copy(
            Vtm[:, 0:H * 64].rearrange("p (h q) -> p h q", q=64)[:, :, 32:64],
            ones_f32[:].rearrange("p (h i) -> p h i", i=32))

        # ---------------- phase 2: attention ----------------
        prev = None
        for h in range(H):
            em0 = em0_first if h == 0 else scores_pair(h, 0)
            if prev is not None:
                attnv_half(prev, 0)
            em1 = scores_pair(h, 1)
            if prev is not None:
                attnv_half(prev, 1)
                finish_norm(prev)
            ps8 = psx.tile([36, NH], F32, tag="ps5")
            prev = (h, [em0, em1], ps8)
        attnv_half(prev, 0)
        attnv_half(prev, 1)
        finish_norm(prev)

        # ---------------- phase 3: output conv + BN + LeakyReLU ----------------
        for u in range(2):
            psy = pss.tile([128, NH], F32, tag="ps")
            nc.tensor.matmul(psy[:], woT_sb[0:64, 128 * u:128 * u + 128], x_sb[:],
                             start=True, stop=True)
            y2 = sp.tile([128, NH], F32, tag="y2")
            nc.vector.tensor_scalar(y2[:], psy[:], s_sb[:, u:u + 1], t_sb[:, u:u + 1],
                                    ALU.mult, ALU.add)
            yt = sp.tile([128, NH], F32, tag="yt")
            nc.vector.scalar_tensor_tensor(yt[:], y2[:], NEG_SLOPE, y2[:],
                                           ALU.mult, ALU.max)
            nc.sync.dma_start(y[128 * u:128 * u + 128, :], yt[:])

        if "dbg_ks" in io:
            nc.sync.dma_start(io["dbg_ks"], Ks[0][:])
            nc.sync.dma_start(io["dbg_vtm"], Vtm[:, 0:H * 64])
            nc.sync.dma_start(io["dbg_x"], x_sb[:])
            nc.sync.dma_start(io["dbg_q"], Qp2[0:100, :])


def build_program(debug_outputs=False):
    nc = bacc.Bacc("TRN2", target_bir_lowering=False, debug=False)
    io = {
        "kb": nc.dram_tensor("kb", [HID, N], BF16, kind="ExternalInput").ap(),
        "qb": nc.dram_tensor("qb", [HID, N], BF16, kind="ExternalInput").ap(),
        "vb": nc.dram_tensor("vb", [HID, N], BF16, kind="ExternalInput").ap(),
        "ebT": nc.dram_tensor("ebT", [H, 128, 8, NH], BF16, kind="ExternalInput").ap(),
        "wkT": nc.dram_tensor("wkT", [HID, 512], BF16, kind="ExternalInput").ap(),
        "wvT": nc.dram_tensor("wvT", [HID, 64], BF16, kind="ExternalInput").ap(),
        "wqT": nc.dram_tensor("wqT", [HID, 32], BF16, kind="ExternalInput").ap(),
        "woT": nc.dram_tensor("woT", [64, HID], F32, kind="ExternalInput").ap(),
        "bnv": nc.dram_tensor("bnv", [128, 10], F32, kind="ExternalInput").ap(),
        "y": nc.dram_tensor("y", [HID, NH], F32, kind="ExternalOutput").ap(),
    }
    if debug_outputs:
        io["dbg_ks"] = nc.dram_tensor("dbg_ks", [128, N], BF16, kind="ExternalOutput").ap()
        io["dbg_vtm"] = nc.dram_tensor("dbg_vtm", [128, H * 64], BF16, kind="ExternalOutput").ap()
        io["dbg_x"] = nc.dram_tensor("dbg_x", [64, NH], F32R, kind="ExternalOutput").ap()
        io["dbg_q"] = nc.dram_tensor("dbg_q", [100, H * NH], BF16, kind="ExternalOutput").ap()
    with tile.TileContext(nc) as tc:
        _emit(nc, tc, io)
    nc.compile()
    return nc


def make_in_maps(q, k, v, attn_bias, Wq, Wk, Wv, Wo, bo, gamma, beta, run_mean, run_var):
    def f32(x):
        return np.ascontiguousarray(np.asarray(x, dtype=np.float32))

    def b16(x):
        return np.ascontiguousarray(np.asarray(x, dtype=np.float32).astype(ml_dtypes.bfloat16))

    q, k, v, attn_bias = f32(q), f32(k), f32(v), f32(attn_bias)
    Wq, Wk, Wv, Wo, bo = f32(Wq), f32(Wk), f32(Wv), f32(Wo), f32(bo)
    gamma, beta, run_mean, run_var = f32(gamma), f32(beta), f32(run_mean), f32(run_var)

    # zero-padded K weight layout: col 32*j + r holds Wk row (j + 16*r)
    # for r < 4, zeros elsewhere -> the M=32 projection matmuls fully
    # initialize their psum row-groups.
    wk3 = np.zeros((HID, 512), dtype=np.float32)
    for j in range(16):
        for r in range(4):
            wk3[:, 32 * j + r] = Wk[j + 16 * r, :]
    wkT = b16(wk3)
    # V weights with cols (c2, d): col 4*c2 + d = Wv row (16*d + c2), so
    # the Vtm gather reads 4-element contiguous runs.
    wv2 = np.empty((HID, 64), dtype=np.float32)
    for c2 in range(16):
        for d in range(4):
            wv2[:, 4 * c2 + d] = Wv[16 * d + c2, :]
    wvT = b16(wv2)
    woT = f32(Wo.T)
    bnv = np.concatenate(
        [x.reshape(2, 128).T for x in (gamma, beta, run_mean, run_var, bo)], axis=1
    )
    bnv = f32(bnv)

    in_maps = []
    for core in range(NCORES):
        b, half = divmod(core, 2)
        n0 = half * NH
        rows = np.array([16 * d + 8 * half + jl for jl in range(8) for d in range(4)])
        wqT = b16(Wq[rows, :].T * SCALE)                          # [256, 32], col = 4*jl+d
        bt = attn_bias[b, :, n0:n0 + NH, :].transpose(0, 2, 1)          # [16, 1024m, 512n]
        ebT = b16(np.exp(bt.reshape(H, 8, 128, NH).transpose(0, 2, 1, 3)))  # [16, 128p, 8t, 512n]
        in_maps.append({
            "kb": b16(k[b]), "qb": b16(q[b]), "vb": b16(v[b]),
            "ebT": ebT, "wkT": wkT, "wvT": wvT, "wqT": wqT, "woT": woT,
            "bnv": bnv,
        })
    return in_maps


_NC_CACHE = None


def get_nc():
    global _NC_CACHE
    if _NC_CACHE is None:
        _NC_CACHE = build_program()
    return _NC_CACHE


def kernel(**inputs):
    nc = get_nc()
    in_maps = make_in_maps(**inputs)
    res = run_bass_kernel_spmd(nc, in_maps, list(range(NCORES)))
    out = np.empty((B, HID, N), dtype=np.float32)
    for core in range(NCORES):
        b, half = divmod(core, 2)
        out[b, :, half * NH:(half + 1) * NH] = res.results[core]["y"]
    return out


# revision 41
# speedup vs baseline: 1.0451x; 1.0021x over previous
"""Trainium2 Bass kernel for nn_MultiHeadAttention_80418967650946.

Reference computation (per batch b):
  qp/kp/vp = 1x1-conv projections of q/k/v   [64, N]
  funky head view: qh[h,n,d] = qp.reshape(4, 16*N)[d, 16n+h]  (same for kh, vh)
  scores = qh @ kh * 0.25^0.5 + bias ; attn = softmax(scores)
  x[4h+d, n] = (attn @ vh)[h, n, d] ; y = LeakyReLU(BN(Wo @ x + bo), 0.2)

Sharding: 8 cores = 4 batches x 2 query-halves (n in [0,512) or [512,1024)).
Each core computes its query-half for ALL 16 heads fully locally (no
collectives): the output conv is column-wise independent, so y[:, n-half]
only needs x[:, n-half].

Per-core device algorithm (all matmul accum fp32):
  - a dummy-matmul warm-up chain at t=0 (overlapped with input DMA) nudges
    the PE HAM clock gate toward 8/8; a dummy exp at t=0 preloads the ACT
    exp table set off the critical path.
  - softmax uses exp(s+b) = exp(s)*exp(b): the host precomputes exp(bias)
    in bf16 (halves HBM traffic vs fp32 bias and removes the f32 psum
    bias-add); the device multiplies exp(scores) by it in the all-bf16
    DVE 2x-rate mode.
  - K projection runs as M=32 matmuls with host-zero-padded weights so
    each psum tile is fully initialized; ONE bulk [100,1024] f32->bf16
    copy per tile stages it into Ks[b4], whose four 32-row groups are
    exactly what the 4-way-packed scores matmuls read (no replication).
  - scores: per [128,1024] psum tile, 4 matmuls (2 key-chunks x 2
    m-halves) pack into distinct PE row/col groups and run concurrently;
    exp() runs on ScalarE psum->sbuf bf16, N=1024 per instruction.
  - attn@V contracts m on partitions via K=128 matmuls whose lhsT window
    over Vtm carries v (cols 0..3) and ones (cols 32..35): psum rows 0..3
    are x, rows 32..35 the softmax denominator.  The denominator must
    reach partition base 0 for DVE (reciprocal_approx_fast mis-executes
    at partition base 32), so a DVE copy + tiny sbuf->sbuf DMA move it.
  - heads are software-pipelined: the PE program alternates scores(h)
    pairs with attn@V(h-1) half-chains so the PE FIFO always has ready
    work and ScalarE stays fed.
"""
import sys

if "/opt/trn_rl_repo" not in sys.path:
    sys.path.insert(0, "/opt/trn_rl_repo")

import numpy as np
import ml_dtypes

import concourse.bass as bass
import concourse.tile as tile
from concourse import bacc, mybir
from concourse.bass_utils import run_bass_kernel_spmd

F32 = mybir.dt.float32
AF = mybir.ActivationFunctionType
ALU = mybir.AluOpType
PSUM = bass.MemorySpace.PSUM
F32R = mybir.dt.float32r
BF16 = mybir.dt.bfloat16


H = 16
D = 4
HID = 256
B = 4
N = 1024
NH = 512          # per-core query positions
NCORES = 8
SCALE = float(D) ** -0.5
BN_EPS = 1e-5
NEG_SLOPE = 0.2


def _emit(nc, tc, io):
    qkvb, wqkv = io["qkvb"], io["wqkv"]
    ebT, woT = io["ebT"], io["woT"]
    bnv, y = io["bnv"], io["y"]

    with (
        tc.tile_pool(name="persist", bufs=1) as persist,
        tc.tile_pool(name="bias", bufs=3) as bp,
        tc.tile_pool(name="exp", bufs=4) as ep,
        tc.tile_pool(name="emul", bufs=4) as em,
        tc.tile_pool(name="sml", bufs=4) as sp,
        tc.tile_pool(name="p1", bufs=1) as p1,
        tc.tile_pool(name="ps_s", bufs=3, space=PSUM) as pss,
        tc.tile_pool(name="ps_x", bufs=2, space=PSUM) as psx,
    ):
        Ks = [persist.tile([128, N], BF16, tag=f"Ks{u}", name=f"Ks{u}")
              for u in range(4)]
        Qp2 = persist.tile([100, H * NH], BF16, tag="Qp2")
        Vtm = persist.tile([128, H * 64 + 96], BF16, tag="Vtm")
        x_sb = persist.tile([64, NH], F32R, tag="x_sb")
        woT_sb = persist.tile([64, HID], F32R, tag="woT_sb")
        s_sb = persist.tile([128, 2], F32, tag="s_sb")
        t_sb = persist.tile([128, 2], F32, tag="t_sb")

        # ---- PE warm-up + ACT table preload: no input deps, issue at t=0.
        wu_w = p1.tile([128, 128], BF16, tag="wu_w")
        wu_r = p1.tile([128, 512], BF16, tag="wu_r")
        nc.vector.memset(wu_w[:], 0.03125)
        nc.vector.memset(wu_r[:], 0.03125)
        scr = p1.tile([128, 8], F32, tag="scr")
        nc.scalar.activation(scr[:], wu_w[:, 0:8], AF.Exp)
        ps_w = pss.tile([128, 512], F32, tag="ps")
        for i in range(8):
            nc.tensor.matmul(ps_w[:], wu_w[:], wu_r[:],
                             start=(i == 0), stop=(i == 7))

        # ---------------- phase 1: input DMAs ----------------
        # weights first (tiny, gate the projections), then one combined
        # q/k/v transfer -- each dma_start costs ~770ns of software-DGE
        # descriptor generation, so fewer is faster.
        w_sb = p1.tile([128, 1216], BF16, tag="w_sb")
        nc.gpsimd.dma_start(w_sb[:].rearrange("p (c o) -> p c o", c=2),
                            wqkv.rearrange("(c p) o -> p c o", p=128))
        qkv_sb = p1.tile([128, 6144], BF16, tag="qkv_sb")
        nc.gpsimd.dma_start(qkv_sb[:].rearrange("p (x c n) -> p x c n", x=3, c=2),
                            qkvb.rearrange("(x c p) n -> p x c n", p=128, c=2))
        nc.gpsimd.dma_start(woT_sb[:], woT)
        bn_sb = p1.tile([128, 10], F32, tag="bn_sb")
        nc.gpsimd.dma_start(bn_sb[:], bnv)
        q_sb = qkv_sb[:, 0:2048]
        k_sb = qkv_sb[:, 2048:4096]
        v_sb = qkv_sb[:, 4096:6144]

        # exp(bias) prefetch: [H, 128, 8, 512] bf16 -> one contiguous 8 KiB
        # read per (partition, head)
        bias_tiles = {}
        for h0 in (0, 2, 4):
            bh2 = bp.tile([128, 8192], BF16, tag="bh2")
            nc.gpsimd.dma_start(
                bh2[:].rearrange("p (h t n) -> p h t n", h=2, t=8),
                ebT[h0:h0 + 2].rearrange("h p t n -> p h t n"))
            bias_tiles[h0] = bh2

        # BN affine: s = gamma * rsqrt(var+eps), t = (bo - mean) * s + beta
        tmp = p1.tile([128, 2], F32, tag="tmp")
        tmp2 = p1.tile([128, 2], F32, tag="tmp2")
        nc.vector.tensor_scalar_add(tmp[:], bn_sb[:, 6:8], BN_EPS)
        nc.scalar.sqrt(tmp[:], tmp[:])
        nc.vector.reciprocal(tmp[:], tmp[:])
        nc.vector.tensor_mul(s_sb[:], bn_sb[:, 0:2], tmp[:])
        nc.vector.tensor_sub(tmp2[:], bn_sb[:, 8:10], bn_sb[:, 4:6])
        nc.vector.tensor_mul(tmp2[:], tmp2[:], s_sb[:])
        nc.vector.tensor_add(t_sb[:], tmp2[:], bn_sb[:, 2:4])

        # ---------------- K projection ----------------
        # M=32 matmuls with host-zero-padded lhsT -> fully-initialized psum,
        # one bulk [100,1024] f32->bf16 copy stages tile b4 into Ks[b4].
        # wk_sb col layout: 512*c2 + 32*j + r  (r<4 real, else 0).
        for b4 in (0, 1):
            psk = pss.tile([128, 1024], F32, tag="ps")
            for g in range(4):
                j = 4 * b4 + g
                for nn2 in range(2):
                    for c in range(2):
                        nc.tensor.matmul(
                            psk[32 * g:32 * g + 32, 512 * nn2:512 * nn2 + 512],
                            w_sb[:, 608 * c + 32 + 32 * j:608 * c + 64 + 32 * j],
                            k_sb[:, 1024 * c + 512 * nn2:1024 * c + 512 * nn2 + 512],
                            start=(c == 0), stop=(c == 1), tile_position=(0, 32 * g))
            if b4 == 1:
                nc.scalar.copy(Ks[b4][0:100, :], psk[0:100, :])
            else:
                nc.vector.tensor_copy(Ks[b4][0:100, :], psk[0:100, :])

        # ---------------- Q projection ----------------
        # 4 j-values col-tiled per [128,1024] psum tile (rows 32g+d hold
        # j = 4*b4+g); SCALE is folded into Wq host-side so the head-major
        # gather into Qp2 is a plain strided copy, split DVE/ScalarE.
        for b4 in range(2):
            psq = pss.tile([128, 1024], F32, tag="ps")
            for g in range(4):
                j = 4 * b4 + g
                for nn2 in range(2):
                    for c in range(2):
                        nc.tensor.matmul(
                            psq[32 * g:32 * g + 4, 512 * nn2:512 * nn2 + 512],
                            w_sb[:, 608 * c + 4 * j:608 * c + 4 * j + 4],
                            q_sb[:, 1024 * c + 512 * nn2:1024 * c + 512 * nn2 + 512],
                            start=(c == 0), stop=(c == 1), tile_position=(0, 32 * g))
            for g in range(4):
                j = 4 * b4 + g
                srcv = psq[32 * g:32 * g + 4, :].rearrange("d (a b) -> d b a", b=16)
                dstv = Qp2[0:4, :].rearrange("d (b q) -> d b q", b=16)[:, :, 64 * j:64 * j + 64]
                if g % 2 == 0:
                    nc.vector.tensor_copy(dstv, srcv)
                else:
                    nc.scalar.copy(dstv, srcv)
        for rep in range(1, 4):
            nc.sync.dma_start(Qp2[32 * rep:32 * rep + 4, :], Qp2[0:4, :])

        # K tiles 2,3 staged after Q (scores(0) needs only Ks0/Ks1+Qp2)
        for b4 in (2, 3):
            psk = pss.tile([128, 1024], F32, tag="ps")
            for g in range(4):
                j = 4 * b4 + g
                for nn2 in range(2):
                    for c in range(2):
                        nc.tensor.matmul(
                            psk[32 * g:32 * g + 32, 512 * nn2:512 * nn2 + 512],
                            w_sb[:, 608 * c + 32 + 32 * j:608 * c + 64 + 32 * j],
                            k_sb[:, 1024 * c + 512 * nn2:1024 * c + 512 * nn2 + 512],
                            start=(c == 0), stop=(c == 1), tile_position=(0, 32 * g))
            if b4 == 1:
                nc.scalar.copy(Ks[b4][0:100, :], psk[0:100, :])
            else:
                nc.vector.tensor_copy(Ks[b4][0:100, :], psk[0:100, :])

        # ---------------- phase 2 stage functions ----------------
        def scores_pair(h, P):
            if P == 0 and h % 2 == 0 and h not in bias_tiles:
                bh2 = bp.tile([128, 8192], BF16, tag="bh2")
                nc.gpsimd.dma_start(
                    bh2[:].rearrange("p (h t n) -> p h t n", h=2, t=8),
                    ebT[h:h + 2].rearrange("h p t n -> p h t n"))
                bias_tiles[h] = bh2
            hb = 4096 * (h % 2)
            bh2 = bias_tiles[h - (h % 2)]
            ex = ep.tile([128, 2048], BF16, tag="ex")
            for uu in range(2):
                u = 2 * P + uu
                # one [128,1024] psum tile = chunks t=2u (cols 0:512) and
                # t=2u+1 (cols 512:); each chunk's two m-halves come from
                # row-groups (2*v2, 2*v2+1) of Ks[u] and land in output
                # partition halves -- all 4 matmuls pack into distinct PE
                # row/col groups and run concurrently.
                ps = pss.tile([128, 1024], F32, tag="ps")
                for v2 in range(2):
                    for mh in range(2):
                        rg = 2 * v2 + mh
                        nc.tensor.matmul(
                            ps[64 * mh:64 * mh + 64, 512 * v2:512 * v2 + 512],
                            Ks[u][32 * rg:32 * rg + 4, h:h + 1009:16],
                            Qp2[32 * rg:32 * rg + 4, 512 * h:512 * h + 512],
                            start=True, stop=True,
                            tile_position=(32 * rg, 64 * mh))
                nc.scalar.activation(ex[:, 1024 * uu:1024 * uu + 1024],
                                     ps[:], AF.Exp)
            # exp(s)*exp(b): all-bf16 contiguous -> DVE 2x-rate mode
            exm = em.tile([128, 2048], BF16, tag="exm")
            nc.vector.tensor_mul(exm[:], ex[:],
                                 bh2[:, hb + 2048 * P:hb + 2048 * P + 2048])
            return exm

        def attnv_half(st, half):
            h, ems, ps8 = st
            for tt in range(4):
                t = 4 * half + tt
                nc.tensor.matmul(
                    ps8[:],
                    Vtm[:, 64 * h + 4 * t:64 * h + 4 * t + 36],
                    ems[half][:, 512 * tt:512 * tt + 512],
                    start=(t == 0), stop=(t == 7))

        def finish_norm(st):
            h, ems, ps8 = st
            d36 = sp.tile([36, NH], F32, tag="d36")
            nc.vector.tensor_copy(d36[:], ps8[:])
            d4 = sp.tile([4, NH], F32, tag="d4")
            nc.sync.dma_start(d4[:], d36[32:36, :])
            r4 = sp.tile([4, NH], F32, tag="r4")
            nc.vector.reciprocal_approx_fast(r4[:], d4[:])
            m4 = sp.tile([4, NH], F32R, tag="m4")
            nc.vector.tensor_mul(m4[:], d36[0:4, :], r4[:])
            nc.sync.dma_start(x_sb[4 * h:4 * h + 4, :], m4[:])

        # first scores pair before the V projection: ScalarE's EXP stream
        # starts as soon as Ks/Qp2 are staged instead of after V.
        em0_first = scores_pair(0, 0)

        # ---------------- V projection ----------------
        # Vtm [128, (h, q64)] bf16:
        #   Vtm[p, 64h + 4t + d]    = vh[m = 128t + p, d]  for head h
        #   Vtm[p, 64h + 32 .. 64]  = 1.0
        # Host supplies wv2 with cols (c2, d) so the per-head gather below
        # reads 4-element contiguous runs; two heads share one [128,64]
        # psum tile (output col-strips 0/64) to double the ring depth.
        nc.vector.memset(Vtm[:], 0.03125)
        for s2 in range(8):
            psv = psx.tile([128, 64], F32, tag="ps5")
            for half in range(2):
                for c in range(2):
                    nc.tensor.matmul(
                        psv[64 * half:64 * half + 64, :],
                        v_sb[:, 1024 * c + 2 * s2 + half:1024 * c + 2 * s2 + half + 1009:16],
                        w_sb[:, 608 * c + 544:608 * c + 608],
                        start=(c == 0), stop=(c == 1),
                        tile_position=(0, 64 * half))
            dst = Vtm[:, 0:H * 64].rearrange("p (h q) -> p h q", q=64)
            for half in range(2):
                s = 2 * s2 + half
                dstv = dst[:, s, 0:32].rearrange("p (t d) -> p t d", d=4)
                pv = psv[64 * half:64 * half + 64, :].rearrange(
                    "r (c2 d) -> r c2 d", d=4)
                nc.vector.tensor_copy(dstv[0:64, :, :], pv[:, 0:16:2, :])
                nc.vector.tensor_copy(dstv[64:128, :, :], pv[:, 1:16:2, :])
        ones_f32 = p1.tile([128, 512], F32, tag="ones_f32")
        nc.vector.memset(ones_f32[:], 1.0)
        nc.vector.tensor_copy(
            Vtm[:, 0:H * 64].rearrange("p (h q) -> p h q", q=64)[:, :, 32:64],
            ones_f32[:].rearrange("p (h i) -> p h i", i=32))

        # ---------------- phase 2: attention ----------------
        prev = None
        for h in range(H):
            em0 = em0_first if h == 0 else scores_pair(h, 0)
            if prev is not None:
                attnv_half(prev, 0)
            em1 = scores_pair(h, 1)
            if prev is not None:
                attnv_half(prev, 1)
                finish_norm(prev)
            ps8 = psx.tile([36, NH], F32, tag="ps5")
            prev = (h, [em0, em1], ps8)
        attnv_half(prev, 0)
        attnv_half(prev, 1)
        finish_norm(prev)

        # ---------------- phase 3: output conv + BN + LeakyReLU ----------------
        for u in range(2):
            psy = pss.tile([128, NH], F32, tag="ps")
            nc.tensor.matmul(psy[:], woT_sb[0:64, 128 * u:128 * u + 128], x_sb[:],
                             start=True, stop=True)
            y2 = sp.tile([128, NH], F32, tag="y2")
            nc.vector.tensor_scalar(y2[:], psy[:], s_sb[:, u:u + 1], t_sb[:, u:u + 1],
                                    ALU.mult, ALU.add)
            yt = sp.tile([128, NH], F32, tag="yt")
            nc.vector.scalar_tensor_tensor(yt[:], y2[:], NEG_SLOPE, y2[:],
                                           ALU.mult, ALU.max)
            nc.sync.dma_start(y[128 * u:128 * u + 128, :], yt[:])

        if "dbg_ks" in io:
            nc.sync.dma_start(io["dbg_ks"], Ks[0][:])
            nc.sync.dma_start(io["dbg_vtm"], Vtm[:, 0:H * 64])
            nc.sync.dma_start(io["dbg_x"], x_sb[:])
            nc.sync.dma_start(io["dbg_q"], Qp2[0:100, :])


def build_program(debug_outputs=False):
    nc = bacc.Bacc("TRN2", target_bir_lowering=False, debug=False)
    io = {
        "qkvb": nc.dram_tensor("qkvb", [3 * HID, N], BF16, kind="ExternalInput").ap(),
        "ebT": nc.dram_tensor("ebT", [H, 128, 8, NH], BF16, kind="ExternalInput").ap(),
        "wqkv": nc.dram_tensor("wqkv", [HID, 608], BF16, kind="ExternalInput").ap(),
        "woT": nc.dram_tensor("woT", [64, HID], F32, kind="ExternalInput").ap(),
        "bnv": nc.dram_tensor("bnv", [128, 10], F32, kind="ExternalInput").ap(),
        "y": nc.dram_tensor("y", [HID, NH], F32, kind="ExternalOutput").ap(),
    }
    if debug_outputs:
        io["dbg_ks"] = nc.dram_tensor("dbg_ks", [128, N], BF16, kind="ExternalOutput").ap()
        io["dbg_vtm"] = nc.dram_tensor("dbg_vtm", [128, H * 64], BF16, kind="ExternalOutput").ap()
        io["dbg_x"] = nc.dram_tensor("dbg_x", [64, NH], F32R, kind="ExternalOutput").ap()
        io["dbg_q"] = nc.dram_tensor("dbg_q", [100, H * NH], BF16, kind="ExternalOutput").ap()
    with tile.TileContext(nc) as tc:
        _emit(nc, tc, io)
    nc.compile()
    return nc


def make_in_maps(q, k, v, attn_bias, Wq, Wk, Wv, Wo, bo, gamma, beta, run_mean, run_var):
    def f32(x):
        return np.ascontiguousarray(np.asarray(x, dtype=np.float32))

    def b16(x):
        return np.ascontiguousarray(np.asarray(x, dtype=np.float32).astype(ml_dtypes.bfloat16))

    q, k, v, attn_bias = f32(q), f32(k), f32(v), f32(attn_bias)
    Wq, Wk, Wv, Wo, bo = f32(Wq), f32(Wk), f32(Wv), f32(Wo), f32(bo)
    gamma, beta, run_mean, run_var = f32(gamma), f32(beta), f32(run_mean), f32(run_var)

    # zero-padded K weight layout: col 32*j + r holds Wk row (j + 16*r)
    # for r < 4, zeros elsewhere -> the M=32 projection matmuls fully
    # initialize their psum row-groups.
    wk3 = np.zeros((HID, 512), dtype=np.float32)
    for j in range(16):
        for r in range(4):
            wk3[:, 32 * j + r] = Wk[j + 16 * r, :]
    # V weights with cols (c2, d): col 4*c2 + d = Wv row (16*d + c2), so
    # the Vtm gather reads 4-element contiguous runs.
    wv2 = np.empty((HID, 64), dtype=np.float32)
    for c2 in range(16):
        for d in range(4):
            wv2[:, 4 * c2 + d] = Wv[16 * d + c2, :]
    woT = f32(Wo.T)
    bnv = np.concatenate(
        [x.reshape(2, 128).T for x in (gamma, beta, run_mean, run_var, bo)], axis=1
    )
    bnv = f32(bnv)

    in_maps = []
    for core in range(NCORES):
        b, half = divmod(core, 2)
        n0 = half * NH
        rows = np.array([16 * d + 8 * half + jl for jl in range(8) for d in range(4)])
        wqT = Wq[rows, :].T * SCALE                               # [256, 32], col = 4*jl+d
        wqkv = b16(np.concatenate([wqT, wk3, wv2], axis=1))       # [256, 608]
        qkvb = b16(np.concatenate([q[b], k[b], v[b]], axis=0))    # [768, 1024]
        bt = attn_bias[b, :, n0:n0 + NH, :].transpose(0, 2, 1)          # [16, 1024m, 512n]
        ebT = b16(np.exp(bt.reshape(H, 8, 128, NH).transpose(0, 2, 1, 3)))  # [16, 128p, 8t, 512n]
        in_maps.append({
            "qkvb": qkvb, "ebT": ebT, "wqkv": wqkv, "woT": woT,
            "bnv": bnv,
        })
    return in_maps


_NC_CACHE = None


def get_nc():
    global _NC_CACHE
    if _NC_CACHE is None:
        _NC_CACHE = build_program()
    return _NC_CACHE


def kernel(**inputs):
    nc = get_nc()
    in_maps = make_in_maps(**inputs)
    res = run_bass_kernel_spmd(nc, in_maps, list(range(NCORES)))
    out = np.empty((B, HID, N), dtype=np.float32)
    for core in range(NCORES):
        b, half = divmod(core, 2)
        out[b, :, half * NH:(half + 1) * NH] = res.results[core]["y"]
    return out
